# revision 20
# baseline (speedup 1.0000x reference)
"""EngagementPredictor TRN2 kernel: 3-branch MHA + masked mean-pool + MLP.

Sharding: pure data-parallel — B=8 batch elements, one per NeuronCore;
weights replicated; no collectives. Each core computes its [2]-logit row.

v10 (fp8 + software pipelining): all projection/attention matmul operands
fp8e4m3 (fp32 PSUM accumulation), DoubleRow packing wherever the
contraction spans >=2 128-tiles (projections, ctx, denominator, tmp/pat
scores) for ~2x PE throughput; end-to-end rel err ~2.9e-3 (gate 2e-2).

Per-core dataflow (S=1024, H=1024):
  xT [H,S] fp8 resident. Per branch (beh 8h / tmp 4h / pat 4h):
    QT,KT [H,S] projections (weights DMA'd once, fp8; Q bias fused into the
    PSUM evacuation; K bias dropped — softmax invariant per-q; V bias folded
    into the pooled vector). V [S,H] natural layout.
    Attention in transposed layout: scoresT[k,q] per (q-chunk 512, head);
    Exp fused with 1/sqrt(d) scale and key mask as per-partition bias.
    Softmax denominator via a ones-matrix lhsT so the [P,512] PSUM result is
    broadcast across partitions; its reciprocal runs on all 128 DVE lanes
    for beh, and as exp(-ln(dn)) on the scalar engine for tmp/pat where
    scalar has slack — the engines are three-way balanced, so the
    reciprocal goes wherever the window has headroom.
    o-projection folded host-side into fus1 (W_m = ow @ fus1_w[m-rows]).
  Emission-order software pipelining: branch m+1's projection groups are
  interleaved between branch m's attention iterations, so the PE's
  attention-phase wait slots (DVE/scalar-paced) are filled with projection
  matmuls. QT/KT/V are double-buffered per branch to allow this.
  Tail: relu MLP (fus2/cls) at M=1.
  Single PSUM pool, tags pj:2 + sc:2 + dn:2 + cx:2 = 8 banks.
"""
import numpy as np
import ml_dtypes

import concourse.bass as bass
import concourse.tile as tile
from concourse import mybir
from concourse.bass_utils import run_bass_kernel_spmd

F32 = mybir.dt.float32
BF16 = mybir.dt.bfloat16
FP8 = mybir.dt.float8e4
DR = mybir.MatmulPerfMode.DoubleRow
AF = mybir.ActivationFunctionType
ALU = mybir.AluOpType

P = 128
S = 1024
H = 1024
NT = H // P          # 8 tiles of 128 along H or S
QC = 512             # projection / psum chunk width
NQC = S // QC        # 2
QCA = 512            # attention q-chunk width
NQCA = S // QCA      # 2
NCORES = 8
MHAS = [("beh", 8), ("tmp", 4), ("pat", 4)]

_CACHE = {}


def _build_nc():
    nc = bass.Bass()
    dram = {}

    def dp(name, shape, dt=BF16):
        dram[name] = nc.declare_dram_parameter(name, list(shape), dt,
                                               isOutput=False)

    dp("xT", (H, S), FP8)
    dp("maskb", (P, NT), F32)  # -30000/0 per key position, partition-inner
    dp("poolwb", (P, S), F32)  # mask/mask_sum, pre-broadcast to 128 rows
    dp("ones", (P, 2, P), FP8)
    for m, _ in MHAS:
        for wn in ("qw", "kw", "vw"):
            dp(f"{m}_{wn}", (H, H), FP8)
        dp(f"{m}_wm", (H, H))
        dp(f"{m}_qb", (P, NT), F32)
        dp(f"{m}_vb", (P, NT), F32)
    dp("b1p", (P, NT), F32)
    dp("fus2_w", (H, H // 2))
    dp("fus2_b", (P, 4), F32)
    dp("cls_w", (H // 2, 2))
    dp("cls_b", (1, 2), F32)
    out = nc.declare_dram_parameter("out", [1, 2], F32, isOutput=True)

    def r3(ap):  # [K, N] dram -> [P, K//P, N] partition-inner
        return ap[:].rearrange("(t p) n -> p t n", p=P)

    with tile.TileContext(nc) as tc, \
         nc.allow_low_precision(
             reason="fp8/bf16 matmul operands with fp32 PSUM accumulation; "
                    "validated at 2.9e-3 rel err vs 2e-2 gate"):
        with tc.tile_pool(name="big", bufs=1) as big, \
             tc.tile_pool(name="wstr", bufs=3) as wstr, \
             tc.tile_pool(name="expp", bufs=3) as expp, \
             tc.tile_pool(name="small", bufs=1) as small, \
             tc.tile_pool(name="work", bufs=3) as work, \
             tc.tile_pool(name="ps", bufs=1, space="PSUM") as ps:

            # ---- resident inputs ----
            xT = big.tile([P, NT, S], FP8, tag="xT")
            nc.sync.dma_start(xT[:, :, 0:QC], r3(dram["xT"])[:, :, 0:QC])
            nc.sync.dma_start(xT[:, :, QC:S], r3(dram["xT"])[:, :, QC:S])

            mb = small.tile([P, NT], F32, tag="mb")
            nc.sync.dma_start(mb[:], dram["maskb"][:])
            pwb = small.tile([P, S], F32, tag="pwb")
            nc.sync.dma_start(pwb[:], dram["poolwb"][:])
            ones = small.tile([P, 2, P], FP8, tag="ones")
            nc.sync.dma_start(ones[:], dram["ones"][:])

            h1acc = small.tile([P, NT], F32, tag="h1acc")
            nc.vector.memset(h1acc[:], 0.0)

            def make_branch(mi, m, nh):
                """Emit DMAs + tile allocs; return (proj_units, attn_units,
                tail_unit) closures for interleaved emission."""
                d = H // nh
                ndt = d // P
                inv_sqrt_d = 1.0 / float(np.sqrt(d))

                # tiny bias tiles FIRST: the Q-evacuation needs qb, and a
                # 4KB DMA queued behind 2MB weight streams stalls the first
                # projection evacuations ~16us at startup
                qb = small.tile([P, NT], F32, tag="qb", bufs=2,
                                name=f"qb{mi}")
                nc.sync.dma_start(qb[:], dram[f"{m}_qb"][:])
                vb = small.tile([P, NT], F32, tag="vb", bufs=2,
                                name=f"vb{mi}")
                nc.sync.dma_start(vb[:], dram[f"{m}_vb"][:])

                qwt = wstr.tile([P, NT, H], FP8, tag="w", bufs=4,
                                name=f"qw{mi}")
                nc.scalar.dma_start(qwt[:, :, 0:QC],
                                    r3(dram[f"{m}_qw"])[:, :, 0:QC])
                nc.scalar.dma_start(qwt[:, :, QC:H],
                                    r3(dram[f"{m}_qw"])[:, :, QC:H])
                kwt = wstr.tile([P, NT, H], FP8, tag="w", bufs=4,
                                name=f"kw{mi}")
                nc.sync.dma_start(kwt[:], r3(dram[f"{m}_kw"]))
                vwt = wstr.tile([P, NT, H], FP8, tag="w", bufs=4,
                                name=f"vw{mi}")
                nc.scalar.dma_start(vwt[:], r3(dram[f"{m}_vw"]))
                wmt = wstr.tile([P, NT, H], BF16, tag="w", bufs=4,
                                name=f"wm{mi}")
                nc.sync.dma_start(wmt[:], r3(dram[f"{m}_wm"]))

                QT = big.tile([P, NT, S], FP8, tag="QT", bufs=2,
                              name=f"QT{mi}")
                KT = big.tile([P, NT, S], FP8, tag="KT", bufs=2,
                              name=f"KT{mi}")
                V = big.tile([P, NT, H], FP8, tag="V", bufs=2,
                             name=f"V{mi}")
                pooled = small.tile([P, NT], F32, tag="pooled", bufs=2,
                                    name=f"pooled{mi}")
                pooledb = small.tile([P, NT], BF16, tag="pooledb", bufs=2,
                                     name=f"pooledb{mi}")

                proj = []

                def qk_group(wt, dst, with_bias, ho, qc, wi):
                    def emit():
                        hsl = slice(ho * P, (ho + 1) * P)
                        qsl = slice(qc * QC, (qc + 1) * QC)
                        pst = ps.tile([P, QC], F32, tag="pj", bufs=2,
                                      name=f"pj{mi}_{wi}_{ho}_{qc}")
                        for ki in range(0, NT, 2):
                            nc.tensor.matmul(
                                pst[:],
                                lhsT=wt[:, ki:ki + 2, hsl],
                                rhs=xT[:, ki:ki + 2, qsl],
                                start=(ki == 0), stop=(ki == NT - 2),
                                perf_mode=DR)
                        if with_bias:
                            nc.scalar.activation(
                                dst[:, ho, qsl], pst[:], AF.Identity,
                                bias=qb[:, ho:ho + 1], scale=1.0)
                        else:
                            nc.vector.tensor_copy(dst[:, ho, qsl], pst[:])
                    return emit

                def v_group(st, hc):
                    def emit():
                        ssl = slice(st * P, (st + 1) * P)
                        hsl = slice(hc * QC, (hc + 1) * QC)
                        pst = ps.tile([P, QC], F32, tag="pj", bufs=2,
                                      name=f"pjv{mi}_{st}_{hc}")
                        for ki in range(0, NT, 2):
                            nc.tensor.matmul(
                                pst[:],
                                lhsT=xT[:, ki:ki + 2, ssl],
                                rhs=vwt[:, ki:ki + 2, hsl],
                                start=(ki == 0), stop=(ki == NT - 2),
                                perf_mode=DR)
                        nc.vector.tensor_copy(V[:, st, hsl], pst[:])
                    return emit

                for wi, (wt, dst, wb_) in enumerate(((qwt, QT, True),
                                                     (kwt, KT, False))):
                    for ho in range(NT):
                        for qc in range(NQC):
                            proj.append(qk_group(wt, dst, wb_, ho, qc, wi))
                for st in range(NT):
                    for hc in range(NQC):
                        proj.append(v_group(st, hc))

                def attn_iter(qc, h):
                    def emit():
                        qsl = slice(qc * QCA, (qc + 1) * QCA)
                        expt = expp.tile([P, NT, QCA], FP8, tag="expt",
                                         name=f"expt{mi}_{qc}_{h}")
                        for kt in range(NT):
                            ksl = slice(kt * P, (kt + 1) * P)
                            ssc = ps.tile([P, QCA], F32, tag="sc", bufs=2,
                                          name=f"sc{mi}_{qc}_{h}_{kt}")
                            if ndt == 2:
                                nc.tensor.matmul(
                                    ssc[:],
                                    lhsT=KT[:, h * 2:h * 2 + 2, ksl],
                                    rhs=QT[:, h * 2:h * 2 + 2, qsl],
                                    start=True, stop=True, perf_mode=DR)
                            else:
                                nc.tensor.matmul(
                                    ssc[:],
                                    lhsT=KT[:, h, ksl],
                                    rhs=QT[:, h, qsl],
                                    start=True, stop=True)
                            nc.scalar.activation(
                                expt[:, kt], ssc[:], AF.Exp,
                                bias=mb[:, kt:kt + 1], scale=inv_sqrt_d)
                        sdn = ps.tile([P, QCA], F32, tag="dn", bufs=2,
                                      name=f"dn{mi}_{qc}_{h}")
                        for kt in range(0, NT, 2):
                            nc.tensor.matmul(
                                sdn[:], lhsT=ones[:],
                                rhs=expt[:, kt:kt + 2],
                                start=(kt == 0), stop=(kt == NT - 2),
                                perf_mode=DR)
                        wb = work.tile([P, QCA], F32, tag="wb")
                        if mi != 1:
                            # beh/pat windows are scalar-exp-paced -> DVE
                            nc.vector.reciprocal(wb[:], sdn[:])
                        else:
                            # tmp window is PE-paced (interleaved pat proj)
                            # and scalar has slack -> 1/dn = exp(-ln dn)
                            lnt = work.tile([P, QCA], F32, tag="lnt")
                            nc.scalar.activation(lnt[:], sdn[:], AF.Ln)
                            nc.scalar.activation(wb[:], lnt[:], AF.Exp,
                                                 scale=-1.0)
                        wbs = work.tile([P, QCA], F32, tag="wbs")
                        nc.vector.tensor_mul(out=wbs[:], in0=wb[:],
                                             in1=pwb[:, qsl])
                        for dt in range(ndt):
                            gdt = h * ndt + dt
                            dsl = slice(gdt * P, (gdt + 1) * P)
                            sctx = ps.tile([P, QCA], F32, tag="cx", bufs=2,
                                           name=f"cx{mi}_{qc}_{h}_{dt}")
                            for kt in range(0, NT, 2):
                                nc.tensor.matmul(
                                    sctx[:], lhsT=V[:, kt:kt + 2, dsl],
                                    rhs=expt[:, kt:kt + 2],
                                    start=(kt == 0), stop=(kt == NT - 2),
                                    perf_mode=DR)
                            prod = work.tile([P, QCA], F32, tag="prod")
                            nc.vector.tensor_mul(out=prod[:], in0=sctx[:],
                                                 in1=wbs[:])
                            if qc == 0:
                                nc.vector.tensor_reduce(
                                    pooled[:, gdt:gdt + 1], prod[:],
                                    axis=mybir.AxisListType.X, op=ALU.add)
                            else:
                                pp = work.tile([P, 1], F32, tag="pp")
                                nc.vector.tensor_reduce(
                                    pp[:], prod[:],
                                    axis=mybir.AxisListType.X, op=ALU.add)
                                nc.vector.tensor_add(
                                    out=pooled[:, gdt:gdt + 1],
                                    in0=pooled[:, gdt:gdt + 1], in1=pp[:])
                    return emit

                attn = [attn_iter(qc, h)
                        for qc in range(NQCA) for h in range(nh)]

                def tail():
                    # + V bias (exact: pooling weights sum to 1)
                    nc.vector.tensor_add(out=pooled[:], in0=pooled[:],
                                         in1=vb[:])
                    nc.vector.tensor_copy(pooledb[:], pooled[:])
                    # fus1 partial (o-proj folded host-side):
                    # h1acc += pooled @ W_m, column layout
                    for tg in range(4):
                        ph1 = [ps.tile([P, 1], F32, tag="cx", bufs=2,
                                       name=f"ph1_{mi}_{tg}_{i}")
                               for i in range(2)]
                        for ki in range(NT):
                            for t2 in range(2):
                                t = tg * 2 + t2
                                nc.tensor.matmul(
                                    ph1[t2][:],
                                    lhsT=wmt[:, ki, t * P:(t + 1) * P],
                                    rhs=pooledb[:, ki:ki + 1],
                                    start=(ki == 0), stop=(ki == NT - 1))
                        for t2 in range(2):
                            t = tg * 2 + t2
                            nc.vector.tensor_add(
                                out=h1acc[:, t:t + 1], in0=ph1[t2][:],
                                in1=h1acc[:, t:t + 1])
                return proj, attn, tail

            # ---- interleaved schedule: proj(m) fills attn(m-1)'s PE gaps
            prev_attn, prev_tail = [], None
            tailc = {}
            for mi, (m, nh) in enumerate(MHAS):
                proj, attn, tail = make_branch(mi, m, nh)
                if mi == 2:
                    # tail constants DMA'd while pat still computes, so the
                    # final MLP chain never waits on transfers
                    tailc["b1"] = small.tile([P, NT], F32, tag="b1", name="b1")
                    nc.sync.dma_start(tailc["b1"][:], dram["b1p"][:])
                    tailc["f2t"] = wstr.tile([P, NT, QC], BF16, tag="w2",
                                             bufs=1, name="f2t")
                    nc.sync.dma_start(tailc["f2t"][:], r3(dram["fus2_w"]))
                    tailc["b2"] = small.tile([P, 4], F32, tag="b2", name="b2")
                    nc.sync.dma_start(tailc["b2"][:], dram["fus2_b"][:])
                    tailc["cwt"] = small.tile([P, 4, 2], BF16, tag="cwt", name="cwt")
                    nc.sync.dma_start(tailc["cwt"][:], r3(dram["cls_w"]))
                    tailc["cb"] = small.tile([1, 2], F32, tag="cb", name="cb")
                    nc.sync.dma_start(tailc["cb"][:], dram["cls_b"][:])
                if not prev_attn:
                    for u in proj:
                        u()
                else:
                    per = (len(proj) + len(prev_attn) - 1) // len(prev_attn)
                    pidx = 0
                    for au in prev_attn:
                        au()
                        for u in proj[pidx:pidx + per]:
                            u()
                        pidx += per
                    for u in proj[pidx:]:
                        u()
                    prev_tail()
                prev_attn, prev_tail = attn, tail
            for au in prev_attn:
                au()
            prev_tail()

            # ---------- MLP tail ----------
            b1 = tailc["b1"]
            h1pre = small.tile([P, NT], F32, tag="h1pre")
            nc.vector.tensor_add(out=h1pre[:], in0=h1acc[:], in1=b1[:])
            h1T = small.tile([P, NT], BF16, tag="h1T")
            nc.scalar.activation(h1T[:], h1pre[:], AF.Relu)

            f2t = tailc["f2t"]
            b2 = tailc["b2"]
            h2T = small.tile([P, 4], BF16, tag="h2T")
            for tg in range(2):
                ph2 = [ps.tile([P, 1], F32, tag="cx", bufs=2,
                               name=f"ph2_{tg}_{i}") for i in range(2)]
                for ki in range(NT):
                    for t2 in range(2):
                        t = tg * 2 + t2
                        nc.tensor.matmul(
                            ph2[t2][:],
                            lhsT=f2t[:, ki, t * P:(t + 1) * P],
                            rhs=h1T[:, ki:ki + 1],
                            start=(ki == 0), stop=(ki == NT - 1))
                for t2 in range(2):
                    t = tg * 2 + t2
                    nc.scalar.activation(h2T[:, t:t + 1], ph2[t2][:],
                                         AF.Relu, bias=b2[:, t:t + 1],
                                         scale=1.0)

            cwt = tailc["cwt"]
            plg = ps.tile([1, 2], F32, tag="dn", bufs=2, name="plg")
            for ki in range(4):
                nc.tensor.matmul(plg[:],
                                 lhsT=h2T[:, ki:ki + 1],
                                 rhs=cwt[:, ki],
                                 start=(ki == 0), stop=(ki == 3))
            cb = tailc["cb"]
            lg = small.tile([1, 2], F32, tag="lgsb")
            nc.vector.tensor_add(out=lg[:], in0=plg[:], in1=cb[:])
            nc.sync.dma_start(out[:], lg[:])

    _split_multi_waits(nc)
    return nc


def _split_multi_waits(nc, max_on_inst=1, max_on_evsem=2):
    """This walrus build caps sync waits per instruction at 1 (2 for
    EventSemaphore); Tile attaches one wait per dependent proc. Spill excess
    waits onto pure-wait EventSemaphores inserted before, on the same engine —
    the engine blocks on each condition in sequence, so semantics match."""
    for f in nc.m.functions:
        for bb in f.blocks:
            insts = list(bb.instructions)
            new = []
            changed = False
            for ins in insts:
                si = ins.sync_info
                if si is not None:
                    waits = list(si.on_wait)
                    cap = (max_on_evsem
                           if isinstance(ins, mybir.InstEventSemaphore)
                           else max_on_inst)
                    if len(waits) > cap:
                        spill = waits[:-cap]
                        keep = waits[-cap:]
                        k = 0
                        while spill:
                            chunk = spill[:max_on_evsem]
                            spill = spill[max_on_evsem:]
                            new.append(mybir.InstEventSemaphore(
                                name=f"{ins.name}-wspill{k}",
                                engine=ins.engine, ins=[], outs=[],
                                sync_info=mybir.SyncInfo(on_wait=chunk,
                                                         on_update=[])))
                            k += 1
                        ins.sync_info = mybir.SyncInfo(
                            on_wait=keep, on_update=list(si.on_update))
                        changed = True
                new.append(ins)
            if changed:
                bb.instructions = new


def _get_nc():
    if "nc" not in _CACHE:
        _CACHE["nc"] = _build_nc()
    return _CACHE["nc"]


def _prep_in_maps(inputs):
    f32 = np.float32
    bf = ml_dtypes.bfloat16
    f8 = ml_dtypes.float8_e4m3

    def pi(v, nt=NT):  # [nt*P] fp32 vector -> [P, nt] partition-inner
        return np.ascontiguousarray(np.asarray(v, f32).reshape(nt, P).T)

    mask = inputs["attention_mask"].astype(f32)          # [B, S]
    denom = mask.sum(axis=1, keepdims=True)              # [B, 1]
    poolw = (mask / denom).astype(f32)                   # [B, S]
    maskb = np.where(mask > 0, 0.0, -30000.0).astype(f32)

    fus1 = inputs["fus1_w"].astype(f32)                  # [3H, H]
    b1p = inputs["fus1_b"].astype(f32)
    shared = {"ones": np.ones((P, 2, P), f8)}
    for mi, (m, _) in enumerate(MHAS):
        for wn in ("qw", "kw", "vw"):
            shared[f"{m}_{wn}"] = np.ascontiguousarray(
                inputs[f"{m}_{wn}"], dtype=f8)
        f1s = fus1[mi * H:(mi + 1) * H]                  # [H, H]
        shared[f"{m}_wm"] = np.ascontiguousarray(
            inputs[f"{m}_ow"].astype(f32) @ f1s, dtype=bf)
        b1p = b1p + inputs[f"{m}_ob"].astype(f32) @ f1s
        shared[f"{m}_qb"] = pi(inputs[f"{m}_qb"])
        shared[f"{m}_vb"] = pi(inputs[f"{m}_vb"])
    shared["b1p"] = pi(b1p)
    shared["fus2_w"] = np.ascontiguousarray(inputs["fus2_w"], dtype=bf)
    shared["fus2_b"] = pi(inputs["fus2_b"], nt=4)
    shared["cls_w"] = np.ascontiguousarray(inputs["cls_w"], dtype=bf)
    shared["cls_b"] = inputs["cls_b"].astype(f32).reshape(1, 2)

    in_maps = []
    for c in range(NCORES):
        im = dict(shared)
        im["xT"] = np.ascontiguousarray(
            inputs["hidden_states"][c].T, dtype=f8)
        im["maskb"] = pi(maskb[c])
        im["poolwb"] = np.ascontiguousarray(
            np.broadcast_to(poolw[c], (P, S)))
        in_maps.append(im)
    return in_maps


def kernel(**inputs) -> np.ndarray:
    nc = _get_nc()
    in_maps = _prep_in_maps(inputs)
    res = run_bass_kernel_spmd(nc, in_maps, core_ids=list(range(NCORES)))
    return np.concatenate(
        [res.results[c]["out"] for c in range(NCORES)], axis=0
    ).astype(np.float32)


# revision 21
# speedup vs baseline: 1.2394x; 1.2394x over previous
"""EngagementPredictor TRN2 kernel: 3-branch MHA + masked mean-pool + MLP.

Sharding: pure data-parallel — B=8 batch elements, one per NeuronCore;
weights replicated; no collectives. Each core computes its [2]-logit row.

v10 (fp8 + software pipelining): all projection/attention matmul operands
fp8e4m3 (fp32 PSUM accumulation), DoubleRow packing wherever the
contraction spans >=2 128-tiles (projections, ctx, denominator, tmp/pat
scores) for ~2x PE throughput; end-to-end rel err ~2.9e-3 (gate 2e-2).

Per-core dataflow (S=1024, H=1024):
  xT [H,S] fp8 resident. Per branch (beh 8h / tmp 4h / pat 4h):
    QT,KT [H,S] projections (weights DMA'd once, fp8; Q bias fused into the
    PSUM evacuation; K bias dropped — softmax invariant per-q; V bias folded
    into the pooled vector). V [S,H] natural layout.
    Attention in transposed layout: scoresT[k,q] per (q-chunk 512, head);
    Exp fused with 1/sqrt(d) scale and key mask as per-partition bias.
    Softmax denominator via a ones-matrix lhsT so the [P,512] PSUM result is
    broadcast across partitions; its reciprocal runs on all 128 DVE lanes
    for beh, and as exp(-ln(dn)) on the scalar engine for tmp/pat where
    scalar has slack — the engines are three-way balanced, so the
    reciprocal goes wherever the window has headroom.
    o-projection folded host-side into fus1 (W_m = ow @ fus1_w[m-rows]).
  Emission-order software pipelining: branch m+1's projection groups are
  interleaved between branch m's attention iterations, so the PE's
  attention-phase wait slots (DVE/scalar-paced) are filled with projection
  matmuls. QT/KT/V are double-buffered per branch to allow this.
  Tail: relu MLP (fus2/cls) at M=1.
  Single PSUM pool, tags pj:2 + sc:2 + dn:2 + cx:2 = 8 banks.
"""
import numpy as np
import ml_dtypes

import concourse.bass as bass
import concourse.tile as tile
from concourse import mybir
from concourse.bass_utils import run_bass_kernel_spmd

F32 = mybir.dt.float32
BF16 = mybir.dt.bfloat16
FP8 = mybir.dt.float8e4
DR = mybir.MatmulPerfMode.DoubleRow
AF = mybir.ActivationFunctionType
ALU = mybir.AluOpType

P = 128
S = 1024
H = 1024
NT = H // P          # 8 tiles of 128 along H or S
QC = 512             # projection / psum chunk width
NQC = S // QC        # 2
QCA = 512            # attention q-chunk width
NQCA = S // QCA      # 2
NCORES = 8
MHAS = [("beh", 8), ("tmp", 4), ("pat", 4)]

_CACHE = {}


def _build_nc():
    nc = bass.Bass()
    dram = {}

    def dp(name, shape, dt=BF16):
        dram[name] = nc.declare_dram_parameter(name, list(shape), dt,
                                               isOutput=False)

    dp("xT", (H, S), FP8)
    dp("maskb", (P, NT), F32)  # -30000/0 per key position, partition-inner
    dp("poolwb", (P, S), F32)  # mask/mask_sum, pre-broadcast to 128 rows
    dp("ones", (P, 2, P), FP8)
    for m, _ in MHAS:
        for wn in ("qw", "kw", "vw"):
            dp(f"{m}_{wn}", (H, H), FP8)
        dp(f"{m}_wm", (H, H))
        dp(f"{m}_qb", (P, NT), F32)
        dp(f"{m}_vb", (P, NT), F32)
    dp("b1p", (P, NT), F32)
    dp("fus2_w", (H, H // 2))
    dp("fus2_b", (P, 4), F32)
    dp("cls_w", (H // 2, 2))
    dp("cls_b", (1, 2), F32)
    out = nc.declare_dram_parameter("out", [1, 2], F32, isOutput=True)

    def r3(ap):  # [K, N] dram -> [P, K//P, N] partition-inner
        return ap[:].rearrange("(t p) n -> p t n", p=P)

    with tile.TileContext(nc) as tc, \
         nc.allow_low_precision(
             reason="fp8/bf16 matmul operands with fp32 PSUM accumulation; "
                    "validated at 2.9e-3 rel err vs 2e-2 gate"):
        with tc.tile_pool(name="big", bufs=1) as big, \
             tc.tile_pool(name="wstr", bufs=3) as wstr, \
             tc.tile_pool(name="expp", bufs=3) as expp, \
             tc.tile_pool(name="small", bufs=1) as small, \
             tc.tile_pool(name="work", bufs=3) as work, \
             tc.tile_pool(name="ps", bufs=1, space="PSUM") as ps:

            # ---- resident inputs ----
            xT = big.tile([P, NT, S], FP8, tag="xT")
            nc.sync.dma_start(xT[:, :, 0:QC], r3(dram["xT"])[:, :, 0:QC])
            nc.sync.dma_start(xT[:, :, QC:S], r3(dram["xT"])[:, :, QC:S])

            mb = small.tile([P, NT], F32, tag="mb")
            nc.sync.dma_start(mb[:], dram["maskb"][:])
            pwb = small.tile([P, S], F32, tag="pwb")
            nc.sync.dma_start(pwb[:], dram["poolwb"][:])
            ones = small.tile([P, 2, P], FP8, tag="ones")
            nc.sync.dma_start(ones[:], dram["ones"][:])

            h1acc = small.tile([P, NT], F32, tag="h1acc")
            nc.vector.memset(h1acc[:], 0.0)

            def make_branch(mi, m, nh):
                """Emit DMAs + tile allocs; return (proj_units, attn_units,
                tail_unit) closures for interleaved emission."""
                d = H // nh
                ndt = d // P
                inv_sqrt_d = 1.0 / float(np.sqrt(d))

                # tiny bias tiles FIRST: the Q-evacuation needs qb, and a
                # 4KB DMA queued behind 2MB weight streams stalls the first
                # projection evacuations ~16us at startup
                qb = small.tile([P, NT], F32, tag="qb", bufs=2,
                                name=f"qb{mi}")
                nc.sync.dma_start(qb[:], dram[f"{m}_qb"][:])
                vb = small.tile([P, NT], F32, tag="vb", bufs=2,
                                name=f"vb{mi}")
                nc.sync.dma_start(vb[:], dram[f"{m}_vb"][:])

                qwt = wstr.tile([P, NT, H], FP8, tag="w", bufs=4,
                                name=f"qw{mi}")
                nc.scalar.dma_start(qwt[:, :, 0:QC],
                                    r3(dram[f"{m}_qw"])[:, :, 0:QC])
                nc.scalar.dma_start(qwt[:, :, QC:H],
                                    r3(dram[f"{m}_qw"])[:, :, QC:H])
                kwt = wstr.tile([P, NT, H], FP8, tag="w", bufs=4,
                                name=f"kw{mi}")
                nc.sync.dma_start(kwt[:], r3(dram[f"{m}_kw"]))
                vwt = wstr.tile([P, NT, H], FP8, tag="w", bufs=4,
                                name=f"vw{mi}")
                nc.scalar.dma_start(vwt[:], r3(dram[f"{m}_vw"]))
                wmt = wstr.tile([P, NT, H], BF16, tag="w", bufs=4,
                                name=f"wm{mi}")
                nc.sync.dma_start(wmt[:], r3(dram[f"{m}_wm"]))

                QT = big.tile([P, NT, S], FP8, tag="QT", bufs=2,
                              name=f"QT{mi}")
                KT = big.tile([P, NT, S], FP8, tag="KT", bufs=2,
                              name=f"KT{mi}")
                V = big.tile([P, NT, H], FP8, tag="V", bufs=2,
                             name=f"V{mi}")
                pooled = small.tile([P, NT], F32, tag="pooled", bufs=2,
                                    name=f"pooled{mi}")
                pooledb = small.tile([P, NT], BF16, tag="pooledb", bufs=2,
                                     name=f"pooledb{mi}")

                proj = []

                def qk_group(wt, dst, with_bias, ho, qc, wi):
                    def emit():
                        hsl = slice(ho * P, (ho + 1) * P)
                        qsl = slice(qc * QC, (qc + 1) * QC)
                        pst = ps.tile([P, QC], F32, tag="pj", bufs=2,
                                      name=f"pj{mi}_{wi}_{ho}_{qc}")
                        for ki in range(0, NT, 2):
                            nc.tensor.matmul(
                                pst[:],
                                lhsT=wt[:, ki:ki + 2, hsl],
                                rhs=xT[:, ki:ki + 2, qsl],
                                start=(ki == 0), stop=(ki == NT - 2),
                                perf_mode=DR)
                        if with_bias:
                            nc.scalar.activation(
                                dst[:, ho, qsl], pst[:], AF.Identity,
                                bias=qb[:, ho:ho + 1], scale=1.0)
                        else:
                            nc.vector.tensor_copy(dst[:, ho, qsl], pst[:])
                    return emit

                def v_group(st, hc):
                    def emit():
                        ssl = slice(st * P, (st + 1) * P)
                        hsl = slice(hc * QC, (hc + 1) * QC)
                        pst = ps.tile([P, QC], F32, tag="pj", bufs=2,
                                      name=f"pjv{mi}_{st}_{hc}")
                        for ki in range(0, NT, 2):
                            nc.tensor.matmul(
                                pst[:],
                                lhsT=xT[:, ki:ki + 2, ssl],
                                rhs=vwt[:, ki:ki + 2, hsl],
                                start=(ki == 0), stop=(ki == NT - 2),
                                perf_mode=DR)
                        nc.vector.tensor_copy(V[:, st, hsl], pst[:])
                    return emit

                for wi, (wt, dst, wb_) in enumerate(((qwt, QT, True),
                                                     (kwt, KT, False))):
                    for ho in range(NT):
                        for qc in range(NQC):
                            proj.append(qk_group(wt, dst, wb_, ho, qc, wi))
                for st in range(NT):
                    for hc in range(NQC):
                        proj.append(v_group(st, hc))

                def attn_iter(qc, h):
                    def emit():
                        qsl = slice(qc * QCA, (qc + 1) * QCA)
                        expt = expp.tile([P, NT, QCA], FP8, tag="expt",
                                         name=f"expt{mi}_{qc}_{h}")
                        for kt in range(NT):
                            ksl = slice(kt * P, (kt + 1) * P)
                            ssc = ps.tile([P, QCA], F32, tag="sc", bufs=2,
                                          name=f"sc{mi}_{qc}_{h}_{kt}")
                            if ndt == 2:
                                nc.tensor.matmul(
                                    ssc[:],
                                    lhsT=KT[:, h * 2:h * 2 + 2, ksl],
                                    rhs=QT[:, h * 2:h * 2 + 2, qsl],
                                    start=True, stop=True, perf_mode=DR)
                            else:
                                nc.tensor.matmul(
                                    ssc[:],
                                    lhsT=KT[:, h, ksl],
                                    rhs=QT[:, h, qsl],
                                    start=True, stop=True)
                            nc.scalar.activation(
                                expt[:, kt], ssc[:], AF.Exp,
                                bias=mb[:, kt:kt + 1], scale=inv_sqrt_d)
                        sdn = ps.tile([P, QCA], F32, tag="dn", bufs=2,
                                      name=f"dn{mi}_{qc}_{h}")
                        for kt in range(0, NT, 2):
                            nc.tensor.matmul(
                                sdn[:], lhsT=ones[:],
                                rhs=expt[:, kt:kt + 2],
                                start=(kt == 0), stop=(kt == NT - 2),
                                perf_mode=DR)
                        wb = work.tile([P, QCA], F32, tag="wb")
                        if mi == 0:
                            # beh: scalar is exp-saturated -> exact DVE recip
                            nc.vector.reciprocal(wb[:], sdn[:])
                        else:
                            # tmp/pat: scalar has slack -> 1/dn = exp(-ln dn)
                            lnt = work.tile([P, QCA], F32, tag="lnt")
                            nc.scalar.activation(lnt[:], sdn[:], AF.Ln)
                            nc.scalar.activation(wb[:], lnt[:], AF.Exp,
                                                 scale=-1.0)
                        wbs = work.tile([P, QCA], F32, tag="wbs")
                        nc.vector.tensor_mul(out=wbs[:], in0=wb[:],
                                             in1=pwb[:, qsl])
                        for dt in range(ndt):
                            gdt = h * ndt + dt
                            dsl = slice(gdt * P, (gdt + 1) * P)
                            sctx = ps.tile([P, QCA], F32, tag="cx", bufs=2,
                                           name=f"cx{mi}_{qc}_{h}_{dt}")
                            for kt in range(0, NT, 2):
                                nc.tensor.matmul(
                                    sctx[:], lhsT=V[:, kt:kt + 2, dsl],
                                    rhs=expt[:, kt:kt + 2],
                                    start=(kt == 0), stop=(kt == NT - 2),
                                    perf_mode=DR)
                            prod = work.tile([P, QCA], F32, tag="prod")
                            nc.vector.tensor_mul(out=prod[:], in0=sctx[:],
                                                 in1=wbs[:])
                            if qc == 0:
                                nc.vector.tensor_reduce(
                                    pooled[:, gdt:gdt + 1], prod[:],
                                    axis=mybir.AxisListType.X, op=ALU.add)
                            else:
                                pp = work.tile([P, 1], F32, tag="pp")
                                nc.vector.tensor_reduce(
                                    pp[:], prod[:],
                                    axis=mybir.AxisListType.X, op=ALU.add)
                                nc.vector.tensor_add(
                                    out=pooled[:, gdt:gdt + 1],
                                    in0=pooled[:, gdt:gdt + 1], in1=pp[:])
                    return emit

                attn = [attn_iter(qc, h)
                        for qc in range(NQCA) for h in range(nh)]

                def tail():
                    # + V bias (exact: pooling weights sum to 1)
                    nc.vector.tensor_add(out=pooled[:], in0=pooled[:],
                                         in1=vb[:])
                    nc.vector.tensor_copy(pooledb[:], pooled[:])
                    # fus1 partial (o-proj folded host-side):
                    # h1acc += pooled @ W_m, column layout
                    for tg in range(4):
                        ph1 = [ps.tile([P, 1], F32, tag="cx", bufs=2,
                                       name=f"ph1_{mi}_{tg}_{i}")
                               for i in range(2)]
                        for ki in range(NT):
                            for t2 in range(2):
                                t = tg * 2 + t2
                                nc.tensor.matmul(
                                    ph1[t2][:],
                                    lhsT=wmt[:, ki, t * P:(t + 1) * P],
                                    rhs=pooledb[:, ki:ki + 1],
                                    start=(ki == 0), stop=(ki == NT - 1))
                        for t2 in range(2):
                            t = tg * 2 + t2
                            nc.vector.tensor_add(
                                out=h1acc[:, t:t + 1], in0=ph1[t2][:],
                                in1=h1acc[:, t:t + 1])
                return proj, attn, tail

            # ---- interleaved schedule: proj(m) fills attn(m-1)'s PE gaps
            prev_attn, prev_tail = [], None
            for mi, (m, nh) in enumerate(MHAS):
                proj, attn, tail = make_branch(mi, m, nh)
                if not prev_attn:
                    for u in proj:
                        u()
                else:
                    per = (len(proj) + len(prev_attn) - 1) // len(prev_attn)
                    pidx = 0
                    for au in prev_attn:
                        au()
                        for u in proj[pidx:pidx + per]:
                            u()
                        pidx += per
                    for u in proj[pidx:]:
                        u()
                    prev_tail()
                prev_attn, prev_tail = attn, tail
            for au in prev_attn:
                au()
            prev_tail()

            # ---------- MLP tail ----------
            b1 = small.tile([P, NT], F32, tag="b1")
            nc.sync.dma_start(b1[:], dram["b1p"][:])
            h1pre = small.tile([P, NT], F32, tag="h1pre")
            nc.vector.tensor_add(out=h1pre[:], in0=h1acc[:], in1=b1[:])
            h1T = small.tile([P, NT], BF16, tag="h1T")
            nc.scalar.activation(h1T[:], h1pre[:], AF.Relu)

            f2t = wstr.tile([P, NT, QC], BF16, tag="w2", bufs=1)
            nc.sync.dma_start(f2t[:], r3(dram["fus2_w"]))
            b2 = small.tile([P, 4], F32, tag="b2")
            nc.sync.dma_start(b2[:], dram["fus2_b"][:])
            h2T = small.tile([P, 4], BF16, tag="h2T")
            for tg in range(2):
                ph2 = [ps.tile([P, 1], F32, tag="cx", bufs=2,
                               name=f"ph2_{tg}_{i}") for i in range(2)]
                for ki in range(NT):
                    for t2 in range(2):
                        t = tg * 2 + t2
                        nc.tensor.matmul(
                            ph2[t2][:],
                            lhsT=f2t[:, ki, t * P:(t + 1) * P],
                            rhs=h1T[:, ki:ki + 1],
                            start=(ki == 0), stop=(ki == NT - 1))
                for t2 in range(2):
                    t = tg * 2 + t2
                    nc.scalar.activation(h2T[:, t:t + 1], ph2[t2][:],
                                         AF.Relu, bias=b2[:, t:t + 1],
                                         scale=1.0)

            cwt = small.tile([P, 4, 2], BF16, tag="cwt")
            nc.sync.dma_start(cwt[:], r3(dram["cls_w"]))
            plg = ps.tile([1, 2], F32, tag="dn", bufs=2, name="plg")
            for ki in range(4):
                nc.tensor.matmul(plg[:],
                                 lhsT=h2T[:, ki:ki + 1],
                                 rhs=cwt[:, ki],
                                 start=(ki == 0), stop=(ki == 3))
            cb = small.tile([1, 2], F32, tag="cb")
            nc.sync.dma_start(cb[:], dram["cls_b"][:])
            lg = small.tile([1, 2], F32, tag="lgsb")
            nc.vector.tensor_add(out=lg[:], in0=plg[:], in1=cb[:])
            nc.sync.dma_start(out[:], lg[:])

    _split_multi_waits(nc)
    return nc


def _split_multi_waits(nc, max_on_inst=1, max_on_evsem=2):
    """This walrus build caps sync waits per instruction at 1 (2 for
    EventSemaphore); Tile attaches one wait per dependent proc. Spill excess
    waits onto pure-wait EventSemaphores inserted before, on the same engine —
    the engine blocks on each condition in sequence, so semantics match."""
    for f in nc.m.functions:
        for bb in f.blocks:
            insts = list(bb.instructions)
            new = []
            changed = False
            for ins in insts:
                si = ins.sync_info
                if si is not None:
                    waits = list(si.on_wait)
                    cap = (max_on_evsem
                           if isinstance(ins, mybir.InstEventSemaphore)
                           else max_on_inst)
                    if len(waits) > cap:
                        spill = waits[:-cap]
                        keep = waits[-cap:]
                        k = 0
                        while spill:
                            chunk = spill[:max_on_evsem]
                            spill = spill[max_on_evsem:]
                            new.append(mybir.InstEventSemaphore(
                                name=f"{ins.name}-wspill{k}",
                                engine=ins.engine, ins=[], outs=[],
                                sync_info=mybir.SyncInfo(on_wait=chunk,
                                                         on_update=[])))
                            k += 1
                        ins.sync_info = mybir.SyncInfo(
                            on_wait=keep, on_update=list(si.on_update))
                        changed = True
                new.append(ins)
            if changed:
                bb.instructions = new


def _get_nc():
    if "nc" not in _CACHE:
        _CACHE["nc"] = _build_nc()
    return _CACHE["nc"]


def _prep_in_maps(inputs):
    f32 = np.float32
    bf = ml_dtypes.bfloat16
    f8 = ml_dtypes.float8_e4m3

    def pi(v, nt=NT):  # [nt*P] fp32 vector -> [P, nt] partition-inner
        return np.ascontiguousarray(np.asarray(v, f32).reshape(nt, P).T)

    mask = inputs["attention_mask"].astype(f32)          # [B, S]
    denom = mask.sum(axis=1, keepdims=True)              # [B, 1]
    poolw = (mask / denom).astype(f32)                   # [B, S]
    maskb = np.where(mask > 0, 0.0, -30000.0).astype(f32)

    fus1 = inputs["fus1_w"].astype(f32)                  # [3H, H]
    b1p = inputs["fus1_b"].astype(f32)
    shared = {"ones": np.ones((P, 2, P), f8)}
    for mi, (m, _) in enumerate(MHAS):
        for wn in ("qw", "kw", "vw"):
            shared[f"{m}_{wn}"] = np.ascontiguousarray(
                inputs[f"{m}_{wn}"], dtype=f8)
        f1s = fus1[mi * H:(mi + 1) * H]                  # [H, H]
        shared[f"{m}_wm"] = np.ascontiguousarray(
            inputs[f"{m}_ow"].astype(f32) @ f1s, dtype=bf)
        b1p = b1p + inputs[f"{m}_ob"].astype(f32) @ f1s
        shared[f"{m}_qb"] = pi(inputs[f"{m}_qb"])
        shared[f"{m}_vb"] = pi(inputs[f"{m}_vb"])
    shared["b1p"] = pi(b1p)
    shared["fus2_w"] = np.ascontiguousarray(inputs["fus2_w"], dtype=bf)
    shared["fus2_b"] = pi(inputs["fus2_b"], nt=4)
    shared["cls_w"] = np.ascontiguousarray(inputs["cls_w"], dtype=bf)
    shared["cls_b"] = inputs["cls_b"].astype(f32).reshape(1, 2)

    in_maps = []
    for c in range(NCORES):
        im = dict(shared)
        im["xT"] = np.ascontiguousarray(
            inputs["hidden_states"][c].T, dtype=f8)
        im["maskb"] = pi(maskb[c])
        im["poolwb"] = np.ascontiguousarray(
            np.broadcast_to(poolw[c], (P, S)))
        in_maps.append(im)
    return in_maps


def kernel(**inputs) -> np.ndarray:
    nc = _get_nc()
    in_maps = _prep_in_maps(inputs)
    res = run_bass_kernel_spmd(nc, in_maps, core_ids=list(range(NCORES)))
    return np.concatenate(
        [res.results[c]["out"] for c in range(NCORES)], axis=0
    ).astype(np.float32)


# revision 22
# speedup vs baseline: 1.2470x; 1.0061x over previous
"""EngagementPredictor TRN2 kernel: 3-branch MHA + masked mean-pool + MLP.

Sharding: pure data-parallel — B=8 batch elements, one per NeuronCore;
weights replicated; no collectives. Each core computes its [2]-logit row.

v10 (fp8 + software pipelining): all projection/attention matmul operands
fp8e4m3 (fp32 PSUM accumulation), DoubleRow packing wherever the
contraction spans >=2 128-tiles (projections, ctx, denominator, tmp/pat
scores) for ~2x PE throughput; end-to-end rel err ~2.9e-3 (gate 2e-2).

Per-core dataflow (S=1024, H=1024):
  xT [H,S] fp8 resident. Per branch (beh 8h / tmp 4h / pat 4h):
    QT,KT [H,S] projections (weights DMA'd once, fp8; Q bias fused into the
    PSUM evacuation; K bias dropped — softmax invariant per-q; V bias folded
    into the pooled vector). V [S,H] natural layout.
    Attention in transposed layout: scoresT[k,q] per (q-chunk 512, head);
    Exp fused with 1/sqrt(d) scale and key mask as per-partition bias.
    Softmax denominator via a ones-matrix lhsT so the [P,512] PSUM result is
    broadcast across partitions; its reciprocal runs on all 128 DVE lanes
    for beh, and as exp(-ln(dn)) on the scalar engine for tmp/pat where
    scalar has slack — the engines are three-way balanced, so the
    reciprocal goes wherever the window has headroom.
    o-projection folded host-side into fus1 (W_m = ow @ fus1_w[m-rows]).
  Emission-order software pipelining: branch m+1's projection groups are
  interleaved between branch m's attention iterations, so the PE's
  attention-phase wait slots (DVE/scalar-paced) are filled with projection
  matmuls. QT/KT/V are double-buffered per branch to allow this.
  Tail: relu MLP (fus2/cls) at M=1.
  Single PSUM pool, tags pj:2 + sc:2 + dn:2 + cx:2 = 8 banks.
"""
import numpy as np
import ml_dtypes

import concourse.bass as bass
import concourse.tile as tile
from concourse import mybir
from concourse.bass_utils import run_bass_kernel_spmd

F32 = mybir.dt.float32
BF16 = mybir.dt.bfloat16
FP8 = mybir.dt.float8e4
DR = mybir.MatmulPerfMode.DoubleRow
AF = mybir.ActivationFunctionType
ALU = mybir.AluOpType

P = 128
S = 1024
H = 1024
NT = H // P          # 8 tiles of 128 along H or S
QC = 512             # projection / psum chunk width
NQC = S // QC        # 2
QCA = 512            # attention q-chunk width
NQCA = S // QCA      # 2
NCORES = 8
MHAS = [("beh", 8), ("tmp", 4), ("pat", 4)]

_CACHE = {}


def _build_nc():
    nc = bass.Bass()
    dram = {}

    def dp(name, shape, dt=BF16):
        dram[name] = nc.declare_dram_parameter(name, list(shape), dt,
                                               isOutput=False)

    dp("xT", (H, S), FP8)
    dp("maskb", (P, NT), F32)  # -30000/0 per key position, partition-inner
    dp("poolwb", (P, S), F32)  # mask/mask_sum, pre-broadcast to 128 rows
    dp("ones", (P, 2, P), FP8)
    for m, _ in MHAS:
        for wn in ("qw", "kw", "vw"):
            dp(f"{m}_{wn}", (H, H), FP8)
        dp(f"{m}_wm", (H, H))
        dp(f"{m}_qb", (P, NT), F32)
        dp(f"{m}_vb", (P, NT), F32)
    dp("b1p", (P, NT), F32)
    dp("fus2_w", (H, H // 2))
    dp("fus2_b", (P, 4), F32)
    dp("cls_w", (H // 2, 2))
    dp("cls_b", (1, 2), F32)
    out = nc.declare_dram_parameter("out", [1, 2], F32, isOutput=True)

    def r3(ap):  # [K, N] dram -> [P, K//P, N] partition-inner
        return ap[:].rearrange("(t p) n -> p t n", p=P)

    with tile.TileContext(nc) as tc, \
         nc.allow_low_precision(
             reason="fp8/bf16 matmul operands with fp32 PSUM accumulation; "
                    "validated at 2.9e-3 rel err vs 2e-2 gate"):
        with tc.tile_pool(name="big", bufs=1) as big, \
             tc.tile_pool(name="wstr", bufs=3) as wstr, \
             tc.tile_pool(name="expp", bufs=3) as expp, \
             tc.tile_pool(name="small", bufs=1) as small, \
             tc.tile_pool(name="work", bufs=3) as work, \
             tc.tile_pool(name="ps", bufs=1, space="PSUM") as ps:

            # ---- resident inputs ----
            xT = big.tile([P, NT, S], FP8, tag="xT")
            nc.sync.dma_start(xT[:, :, 0:QC], r3(dram["xT"])[:, :, 0:QC])
            nc.sync.dma_start(xT[:, :, QC:S], r3(dram["xT"])[:, :, QC:S])

            mb = small.tile([P, NT], F32, tag="mb")
            nc.sync.dma_start(mb[:], dram["maskb"][:])
            pwb = small.tile([P, S], F32, tag="pwb")
            nc.sync.dma_start(pwb[:], dram["poolwb"][:])
            ones = small.tile([P, 2, P], FP8, tag="ones")
            nc.sync.dma_start(ones[:], dram["ones"][:])

            h1acc = small.tile([P, NT], F32, tag="h1acc")
            nc.vector.memset(h1acc[:], 0.0)

            def make_branch(mi, m, nh):
                """Emit DMAs + tile allocs; return (proj_units, attn_units,
                tail_unit) closures for interleaved emission."""
                d = H // nh
                ndt = d // P
                inv_sqrt_d = 1.0 / float(np.sqrt(d))

                # tiny bias tiles FIRST: the Q-evacuation needs qb, and a
                # 4KB DMA queued behind 2MB weight streams stalls the first
                # projection evacuations ~16us at startup
                qb = small.tile([P, NT], F32, tag="qb", bufs=2,
                                name=f"qb{mi}")
                nc.sync.dma_start(qb[:], dram[f"{m}_qb"][:])
                vb = small.tile([P, NT], F32, tag="vb", bufs=2,
                                name=f"vb{mi}")
                nc.sync.dma_start(vb[:], dram[f"{m}_vb"][:])

                qwt = wstr.tile([P, NT, H], FP8, tag="w", bufs=4,
                                name=f"qw{mi}")
                nc.scalar.dma_start(qwt[:, :, 0:QC],
                                    r3(dram[f"{m}_qw"])[:, :, 0:QC])
                nc.scalar.dma_start(qwt[:, :, QC:H],
                                    r3(dram[f"{m}_qw"])[:, :, QC:H])
                kwt = wstr.tile([P, NT, H], FP8, tag="w", bufs=4,
                                name=f"kw{mi}")
                nc.sync.dma_start(kwt[:], r3(dram[f"{m}_kw"]))
                vwt = wstr.tile([P, NT, H], FP8, tag="w", bufs=4,
                                name=f"vw{mi}")
                nc.scalar.dma_start(vwt[:], r3(dram[f"{m}_vw"]))
                wmt = wstr.tile([P, NT, H], BF16, tag="w", bufs=4,
                                name=f"wm{mi}")
                nc.sync.dma_start(wmt[:], r3(dram[f"{m}_wm"]))

                QT = big.tile([P, NT, S], FP8, tag="QT", bufs=2,
                              name=f"QT{mi}")
                KT = big.tile([P, NT, S], FP8, tag="KT", bufs=2,
                              name=f"KT{mi}")
                V = big.tile([P, NT, H], FP8, tag="V", bufs=2,
                             name=f"V{mi}")
                pooled = small.tile([P, NT], F32, tag="pooled", bufs=2,
                                    name=f"pooled{mi}")
                pooledb = small.tile([P, NT], BF16, tag="pooledb", bufs=2,
                                     name=f"pooledb{mi}")

                proj = []

                def qk_group(wt, dst, with_bias, ho, qc, wi):
                    def emit():
                        hsl = slice(ho * P, (ho + 1) * P)
                        qsl = slice(qc * QC, (qc + 1) * QC)
                        pst = ps.tile([P, QC], F32, tag="pj", bufs=2,
                                      name=f"pj{mi}_{wi}_{ho}_{qc}")
                        for ki in range(0, NT, 2):
                            nc.tensor.matmul(
                                pst[:],
                                lhsT=wt[:, ki:ki + 2, hsl],
                                rhs=xT[:, ki:ki + 2, qsl],
                                start=(ki == 0), stop=(ki == NT - 2),
                                perf_mode=DR)
                        if with_bias:
                            nc.scalar.activation(
                                dst[:, ho, qsl], pst[:], AF.Identity,
                                bias=qb[:, ho:ho + 1], scale=1.0)
                        elif ho % 2 == 0:
                            # split K evacs across both engines: these run
                            # interleaved inside the prior branch's attention
                            # window, where scalar (exp) and DVE (recip/pool)
                            # are both near-saturated
                            nc.scalar.activation(dst[:, ho, qsl], pst[:],
                                                 AF.Identity)
                        else:
                            nc.vector.tensor_copy(dst[:, ho, qsl], pst[:])
                    return emit

                def v_group(st, hc):
                    def emit():
                        ssl = slice(st * P, (st + 1) * P)
                        hsl = slice(hc * QC, (hc + 1) * QC)
                        pst = ps.tile([P, QC], F32, tag="pj", bufs=2,
                                      name=f"pjv{mi}_{st}_{hc}")
                        for ki in range(0, NT, 2):
                            nc.tensor.matmul(
                                pst[:],
                                lhsT=xT[:, ki:ki + 2, ssl],
                                rhs=vwt[:, ki:ki + 2, hsl],
                                start=(ki == 0), stop=(ki == NT - 2),
                                perf_mode=DR)
                        nc.vector.tensor_copy(V[:, st, hsl], pst[:])
                    return emit

                for wi, (wt, dst, wb_) in enumerate(((qwt, QT, True),
                                                     (kwt, KT, False))):
                    for ho in range(NT):
                        for qc in range(NQC):
                            proj.append(qk_group(wt, dst, wb_, ho, qc, wi))
                for st in range(NT):
                    for hc in range(NQC):
                        proj.append(v_group(st, hc))

                def attn_iter(qc, h):
                    def emit():
                        qsl = slice(qc * QCA, (qc + 1) * QCA)
                        expt = expp.tile([P, NT, QCA], FP8, tag="expt",
                                         name=f"expt{mi}_{qc}_{h}")
                        for kt in range(NT):
                            ksl = slice(kt * P, (kt + 1) * P)
                            ssc = ps.tile([P, QCA], F32, tag="sc", bufs=2,
                                          name=f"sc{mi}_{qc}_{h}_{kt}")
                            if ndt == 2:
                                nc.tensor.matmul(
                                    ssc[:],
                                    lhsT=KT[:, h * 2:h * 2 + 2, ksl],
                                    rhs=QT[:, h * 2:h * 2 + 2, qsl],
                                    start=True, stop=True, perf_mode=DR)
                            else:
                                nc.tensor.matmul(
                                    ssc[:],
                                    lhsT=KT[:, h, ksl],
                                    rhs=QT[:, h, qsl],
                                    start=True, stop=True)
                            nc.scalar.activation(
                                expt[:, kt], ssc[:], AF.Exp,
                                bias=mb[:, kt:kt + 1], scale=inv_sqrt_d)
                        sdn = ps.tile([P, QCA], F32, tag="dn", bufs=2,
                                      name=f"dn{mi}_{qc}_{h}")
                        for kt in range(0, NT, 2):
                            nc.tensor.matmul(
                                sdn[:], lhsT=ones[:],
                                rhs=expt[:, kt:kt + 2],
                                start=(kt == 0), stop=(kt == NT - 2),
                                perf_mode=DR)
                        wb = work.tile([P, QCA], F32, tag="wb")
                        if mi == 0:
                            # beh: scalar is exp-saturated -> exact DVE recip
                            nc.vector.reciprocal(wb[:], sdn[:])
                        else:
                            # tmp/pat: scalar has slack -> 1/dn = exp(-ln dn)
                            lnt = work.tile([P, QCA], F32, tag="lnt")
                            nc.scalar.activation(lnt[:], sdn[:], AF.Ln)
                            nc.scalar.activation(wb[:], lnt[:], AF.Exp,
                                                 scale=-1.0)
                        wbs = work.tile([P, QCA], F32, tag="wbs")
                        nc.vector.tensor_mul(out=wbs[:], in0=wb[:],
                                             in1=pwb[:, qsl])
                        for dt in range(ndt):
                            gdt = h * ndt + dt
                            dsl = slice(gdt * P, (gdt + 1) * P)
                            sctx = ps.tile([P, QCA], F32, tag="cx", bufs=2,
                                           name=f"cx{mi}_{qc}_{h}_{dt}")
                            for kt in range(0, NT, 2):
                                nc.tensor.matmul(
                                    sctx[:], lhsT=V[:, kt:kt + 2, dsl],
                                    rhs=expt[:, kt:kt + 2],
                                    start=(kt == 0), stop=(kt == NT - 2),
                                    perf_mode=DR)
                            prod = work.tile([P, QCA], F32, tag="prod")
                            nc.vector.tensor_mul(out=prod[:], in0=sctx[:],
                                                 in1=wbs[:])
                            if qc == 0:
                                nc.vector.tensor_reduce(
                                    pooled[:, gdt:gdt + 1], prod[:],
                                    axis=mybir.AxisListType.X, op=ALU.add)
                            else:
                                pp = work.tile([P, 1], F32, tag="pp")
                                nc.vector.tensor_reduce(
                                    pp[:], prod[:],
                                    axis=mybir.AxisListType.X, op=ALU.add)
                                nc.vector.tensor_add(
                                    out=pooled[:, gdt:gdt + 1],
                                    in0=pooled[:, gdt:gdt + 1], in1=pp[:])
                    return emit

                attn = [attn_iter(qc, h)
                        for qc in range(NQCA) for h in range(nh)]

                def tail():
                    # + V bias (exact: pooling weights sum to 1)
                    nc.vector.tensor_add(out=pooled[:], in0=pooled[:],
                                         in1=vb[:])
                    nc.vector.tensor_copy(pooledb[:], pooled[:])
                    # fus1 partial (o-proj folded host-side):
                    # h1acc += pooled @ W_m, column layout
                    for tg in range(4):
                        ph1 = [ps.tile([P, 1], F32, tag="cx", bufs=2,
                                       name=f"ph1_{mi}_{tg}_{i}")
                               for i in range(2)]
                        for ki in range(NT):
                            for t2 in range(2):
                                t = tg * 2 + t2
                                nc.tensor.matmul(
                                    ph1[t2][:],
                                    lhsT=wmt[:, ki, t * P:(t + 1) * P],
                                    rhs=pooledb[:, ki:ki + 1],
                                    start=(ki == 0), stop=(ki == NT - 1))
                        for t2 in range(2):
                            t = tg * 2 + t2
                            nc.vector.tensor_add(
                                out=h1acc[:, t:t + 1], in0=ph1[t2][:],
                                in1=h1acc[:, t:t + 1])
                return proj, attn, tail

            # ---- interleaved schedule: proj(m) fills attn(m-1)'s PE gaps
            prev_attn, prev_tail = [], None
            for mi, (m, nh) in enumerate(MHAS):
                proj, attn, tail = make_branch(mi, m, nh)
                if not prev_attn:
                    for u in proj:
                        u()
                else:
                    per = (len(proj) + len(prev_attn) - 1) // len(prev_attn)
                    pidx = 0
                    for au in prev_attn:
                        au()
                        for u in proj[pidx:pidx + per]:
                            u()
                        pidx += per
                    for u in proj[pidx:]:
                        u()
                    prev_tail()
                prev_attn, prev_tail = attn, tail
            for au in prev_attn:
                au()
            prev_tail()

            # ---------- MLP tail ----------
            b1 = small.tile([P, NT], F32, tag="b1")
            nc.sync.dma_start(b1[:], dram["b1p"][:])
            h1pre = small.tile([P, NT], F32, tag="h1pre")
            nc.vector.tensor_add(out=h1pre[:], in0=h1acc[:], in1=b1[:])
            h1T = small.tile([P, NT], BF16, tag="h1T")
            nc.scalar.activation(h1T[:], h1pre[:], AF.Relu)

            f2t = wstr.tile([P, NT, QC], BF16, tag="w2", bufs=1)
            nc.sync.dma_start(f2t[:], r3(dram["fus2_w"]))
            b2 = small.tile([P, 4], F32, tag="b2")
            nc.sync.dma_start(b2[:], dram["fus2_b"][:])
            h2T = small.tile([P, 4], BF16, tag="h2T")
            for tg in range(2):
                ph2 = [ps.tile([P, 1], F32, tag="cx", bufs=2,
                               name=f"ph2_{tg}_{i}") for i in range(2)]
                for ki in range(NT):
                    for t2 in range(2):
                        t = tg * 2 + t2
                        nc.tensor.matmul(
                            ph2[t2][:],
                            lhsT=f2t[:, ki, t * P:(t + 1) * P],
                            rhs=h1T[:, ki:ki + 1],
                            start=(ki == 0), stop=(ki == NT - 1))
                for t2 in range(2):
                    t = tg * 2 + t2
                    nc.scalar.activation(h2T[:, t:t + 1], ph2[t2][:],
                                         AF.Relu, bias=b2[:, t:t + 1],
                                         scale=1.0)

            cwt = small.tile([P, 4, 2], BF16, tag="cwt")
            nc.sync.dma_start(cwt[:], r3(dram["cls_w"]))
            plg = ps.tile([1, 2], F32, tag="dn", bufs=2, name="plg")
            for ki in range(4):
                nc.tensor.matmul(plg[:],
                                 lhsT=h2T[:, ki:ki + 1],
                                 rhs=cwt[:, ki],
                                 start=(ki == 0), stop=(ki == 3))
            cb = small.tile([1, 2], F32, tag="cb")
            nc.sync.dma_start(cb[:], dram["cls_b"][:])
            lg = small.tile([1, 2], F32, tag="lgsb")
            nc.vector.tensor_add(out=lg[:], in0=plg[:], in1=cb[:])
            nc.sync.dma_start(out[:], lg[:])

    _split_multi_waits(nc)
    return nc


def _split_multi_waits(nc, max_on_inst=1, max_on_evsem=2):
    """This walrus build caps sync waits per instruction at 1 (2 for
    EventSemaphore); Tile attaches one wait per dependent proc. Spill excess
    waits onto pure-wait EventSemaphores inserted before, on the same engine —
    the engine blocks on each condition in sequence, so semantics match."""
    for f in nc.m.functions:
        for bb in f.blocks:
            insts = list(bb.instructions)
            new = []
            changed = False
            for ins in insts:
                si = ins.sync_info
                if si is not None:
                    waits = list(si.on_wait)
                    cap = (max_on_evsem
                           if isinstance(ins, mybir.InstEventSemaphore)
                           else max_on_inst)
                    if len(waits) > cap:
                        spill = waits[:-cap]
                        keep = waits[-cap:]
                        k = 0
                        while spill:
                            chunk = spill[:max_on_evsem]
                            spill = spill[max_on_evsem:]
                            new.append(mybir.InstEventSemaphore(
                                name=f"{ins.name}-wspill{k}",
                                engine=ins.engine, ins=[], outs=[],
                                sync_info=mybir.SyncInfo(on_wait=chunk,
                                                         on_update=[])))
                            k += 1
                        ins.sync_info = mybir.SyncInfo(
                            on_wait=keep, on_update=list(si.on_update))
                        changed = True
                new.append(ins)
            if changed:
                bb.instructions = new


def _get_nc():
    if "nc" not in _CACHE:
        _CACHE["nc"] = _build_nc()
    return _CACHE["nc"]


def _prep_in_maps(inputs):
    f32 = np.float32
    bf = ml_dtypes.bfloat16
    f8 = ml_dtypes.float8_e4m3

    def pi(v, nt=NT):  # [nt*P] fp32 vector -> [P, nt] partition-inner
        return np.ascontiguousarray(np.asarray(v, f32).reshape(nt, P).T)

    mask = inputs["attention_mask"].astype(f32)          # [B, S]
    denom = mask.sum(axis=1, keepdims=True)              # [B, 1]
    poolw = (mask / denom).astype(f32)                   # [B, S]
    maskb = np.where(mask > 0, 0.0, -30000.0).astype(f32)

    fus1 = inputs["fus1_w"].astype(f32)                  # [3H, H]
    b1p = inputs["fus1_b"].astype(f32)
    shared = {"ones": np.ones((P, 2, P), f8)}
    for mi, (m, _) in enumerate(MHAS):
        for wn in ("qw", "kw", "vw"):
            shared[f"{m}_{wn}"] = np.ascontiguousarray(
                inputs[f"{m}_{wn}"], dtype=f8)
        f1s = fus1[mi * H:(mi + 1) * H]                  # [H, H]
        shared[f"{m}_wm"] = np.ascontiguousarray(
            inputs[f"{m}_ow"].astype(f32) @ f1s, dtype=bf)
        b1p = b1p + inputs[f"{m}_ob"].astype(f32) @ f1s
        shared[f"{m}_qb"] = pi(inputs[f"{m}_qb"])
        shared[f"{m}_vb"] = pi(inputs[f"{m}_vb"])
    shared["b1p"] = pi(b1p)
    shared["fus2_w"] = np.ascontiguousarray(inputs["fus2_w"], dtype=bf)
    shared["fus2_b"] = pi(inputs["fus2_b"], nt=4)
    shared["cls_w"] = np.ascontiguousarray(inputs["cls_w"], dtype=bf)
    shared["cls_b"] = inputs["cls_b"].astype(f32).reshape(1, 2)

    in_maps = []
    for c in range(NCORES):
        im = dict(shared)
        im["xT"] = np.ascontiguousarray(
            inputs["hidden_states"][c].T, dtype=f8)
        im["maskb"] = pi(maskb[c])
        im["poolwb"] = np.ascontiguousarray(
            np.broadcast_to(poolw[c], (P, S)))
        in_maps.append(im)
    return in_maps


def kernel(**inputs) -> np.ndarray:
    nc = _get_nc()
    in_maps = _prep_in_maps(inputs)
    res = run_bass_kernel_spmd(nc, in_maps, core_ids=list(range(NCORES)))
    return np.concatenate(
        [res.results[c]["out"] for c in range(NCORES)], axis=0
    ).astype(np.float32)
